# revision 2
# baseline (speedup 1.0000x reference)
"""Trainium2 Bass kernel for nn_Conv2D_80796924772741.

Depthwise (grouped, F=64) 3x3 valid conv over [F, 514, 514, 4] int8 with
per-channel int8 weights + int32 bias, followed by exact fixed-point requant
  res = (acc * 19920 + 2^21) >> 22 ;  out = clip(res - 5, -128, 127) int8
(reduced_mantissa 19920 = 1245 * 16 -> res = (acc*1245 + 2^17) >> 18).

Sharding: F=64 split across 8 NeuronCores (8 channels each), embarrassingly
parallel.

Per-core compute:
 - PE: per channel, conv via Toeplitz-band stationary matmuls over H-windows
   (contraction = 128 input rows; all 3 H-taps in the band diagonals; 3
   matmuls for the 3 W-taps, W-shift = +4n free-dim offset since (w,d) is
   flattened). Bias b and a -63.5 rounding offset ride two all-ones rhs
   partitions with per-output-column weights.  PSUM accA = conv + b - 63.5.
 - ACT: h'' = fma(accA * 2^-7 + 1.5*2^23)  == 1.5*2^23 + floor(acc/128)
   (exact: RNE at ulp=1, offset -63.5 centers the fraction, never ties).
 - DVE: hi = h'' - 1.5*2^23  (exact, fits fp16)
 - PE: accA += (-128*I) @ hi  -> l' = lo - 63.5  (lo = acc mod 128)
 - ACT: g = fma(l' * (1245/128) - 8598.861328125)   [= gamma - c2 - 9216]
 - DVE: f5 = RNE(g)          (magic-add pair)      [= floor(gamma) - 9216]
        S  = hi*1245 + f5    (scalar_tensor_tensor; exact, < 2^21)
        v  = S*2^-11 - 0.499755859375
        r  = RNE(v)                                 [= res - 5]
        out = clip(r, -128, 127) -> int8
Every intermediate is exactly representable in fp32; the chain was verified
bit-exact against the int64 reference over the full accumulator range.
"""

import numpy as np
import ml_dtypes

F_PER_CORE = 8
H_IN = 514
W_IN = 514
D = 4
H_OUT = 512
WD_OUT = 2048  # 512 * 4
FREE_IN = W_IN * D  # 2056
N_CHUNK = 512
N_CORES = 8

# H windows: output rows per window (partition-limited: K = M + 4 <= 128)
WINDOWS = [(0, 124), (124, 124), (248, 124), (372, 124), (496, 16)]

MAGIC = 12582912.0  # 1.5 * 2^23 : RNE-to-integer magic for |x| < 2^22


def _build_lhsT(w_core: np.ndarray, b_core: np.ndarray) -> np.ndarray:
    """[128, 8*3*124] bf16 stationary: per (channel, w-tap) a Toeplitz band.

    Layout column block (f*3 + n)*124 : +124  holds T_n for channel f.
    T_n[2 + i + m, i] = w[f, m, n]  (rows 2.. are conv data partitions)
    T_0[0, i] = 8*floor(b/8) ; T_0[1, i] = (b mod 8) - 63.5  (bias rows,
    multiplied by all-ones rhs partitions 0/1).
    """
    out = np.zeros((128, F_PER_CORE * 3 * 124), dtype=np.float32)
    for f in range(F_PER_CORE):
        b_f = int(b_core[f])
        bh = b_f >> 3  # floor division
        bl = b_f - 8 * bh
        for n in range(3):
            base = (f * 3 + n) * 124
            if n == 0:
                out[0, base : base + 124] = float(8 * bh)
                out[1, base : base + 124] = float(bl) - 63.5
            for m in range(3):
                wv = float(int(w_core[f, m, n, 0]))
                # T[2 + i + m, base + i] = wv  for i in 0..123
                idx = np.arange(124)
                out[2 + idx + m, base + idx] = wv
    return out.astype(ml_dtypes.bfloat16)


_PROGRAM_CACHE = {}


def _build_program():
    import concourse.bass as bass
    import concourse.tile as tile
    from concourse import bacc, mybir

    nc = bacc.Bacc(
        "TRN2", target_bir_lowering=False, debug=False, num_devices=N_CORES
    )
    dt = mybir.dt
    Alu = mybir.AluOpType
    Act = mybir.ActivationFunctionType

    x_d = nc.dram_tensor(
        "x", [F_PER_CORE, H_IN, FREE_IN], dt.int8, kind="ExternalInput"
    ).ap()
    lhsT_d = nc.dram_tensor(
        "lhsT", [128, F_PER_CORE * 3 * 124], dt.bfloat16, kind="ExternalInput"
    ).ap()
    id_d = nc.dram_tensor("id4", [124, 124], dt.float16, kind="ExternalInput").ap()
    ones_d = nc.dram_tensor("ones2", [2, FREE_IN], dt.bfloat16, kind="ExternalInput").ap()
    y_d = nc.dram_tensor(
        "y", [F_PER_CORE, H_OUT, WD_OUT], dt.int8, kind="ExternalOutput"
    ).ap()

    with tile.TileContext(nc) as tc:
        with (
            tc.tile_pool(name="const", bufs=1) as const_pool,
            tc.tile_pool(name="xin", bufs=3) as x_pool,
            tc.tile_pool(name="psum", bufs=6, space="PSUM") as psum_pool,
            tc.tile_pool(name="hbig", bufs=3) as h_pool,
            tc.tile_pool(name="hi16", bufs=3) as hi_pool,
            tc.tile_pool(name="gtile", bufs=3) as g_pool,
            tc.tile_pool(name="ftile", bufs=3) as f_pool,
            tc.tile_pool(name="stile", bufs=3) as s_pool,
            tc.tile_pool(name="vtile", bufs=3) as v_pool,
            tc.tile_pool(name="rtile", bufs=3) as r_pool,
            tc.tile_pool(name="otile", bufs=3) as o_pool,
        ):
            lhsT_t = const_pool.tile([128, F_PER_CORE * 3 * 124], dt.bfloat16)
            nc.sync.dma_start(lhsT_t[:], lhsT_d[:])
            id_t = const_pool.tile([124, 124], dt.float16)
            nc.sync.dma_start(id_t[:], id_d[:])

            for f in range(F_PER_CORE):
                for (r0, m_r) in WINDOWS:
                    k_r = m_r + 4  # 2 ones rows + m_r + 2 data rows
                    xt = x_pool.tile([128, FREE_IN], dt.bfloat16)
                    # ones rows (bias partitions)
                    nc.sync.dma_start(xt[0:2, :], ones_d[:])
                    # data rows with int8 -> bf16 cast (SWDGE)
                    nc.gpsimd.dma_start(
                        xt[2 : 2 + m_r + 2, :], x_d[f, r0 : r0 + m_r + 2, :]
                    )
                    for c in range(4):
                        ps = psum_pool.tile([124, N_CHUNK], dt.float32)
                        for n in range(3):
                            base = (f * 3 + n) * 124
                            nc.tensor.matmul(
                                ps[0:m_r, :],
                                lhsT_t[0:k_r, base : base + m_r],
                                xt[0:k_r, c * N_CHUNK + 4 * n : c * N_CHUNK + 4 * n + N_CHUNK],
                                start=(n == 0),
                                stop=False,
                                skip_group_check=True,
                            )
                        # h'' = 1.5*2^23 + floor(acc/128)
                        ht = h_pool.tile([124, N_CHUNK], dt.float32)
                        nc.scalar.activation(
                            ht[0:m_r, :], ps[0:m_r, :], Act.Copy,
                            bias=MAGIC, scale=0.0078125,
                        )
                        hit = hi_pool.tile([124, N_CHUNK], dt.float16)
                        nc.vector.tensor_scalar(
                            hit[0:m_r, :], ht[0:m_r, :], -MAGIC, None, Alu.add
                        )
                        # accA += -128 * hi  -> l' = (acc mod 128) - 63.5
                        nc.tensor.matmul(
                            ps[0:m_r, :],
                            id_t[0:m_r, 0:m_r],
                            hit[0:m_r, :],
                            start=False,
                            stop=True,
                            skip_group_check=True,
                        )
                        # g = gamma - c2 - 9216
                        gt = g_pool.tile([124, N_CHUNK], dt.float32)
                        nc.scalar.activation(
                            gt[0:m_r, :], ps[0:m_r, :], Act.Copy,
                            bias=-8598.861328125, scale=9.7265625,
                        )
                        # f5 = RNE(g) = floor(gamma) - 9216
                        ft = f_pool.tile([124, N_CHUNK], dt.float32)
                        nc.vector.tensor_scalar(
                            ft[0:m_r, :], gt[0:m_r, :], MAGIC, -MAGIC, Alu.add, Alu.add
                        )
                        # S = hi*1245 + f5
                        st = s_pool.tile([124, N_CHUNK], dt.float32)
                        nc.vector.scalar_tensor_tensor(
                            st[0:m_r, :], hit[0:m_r, :], 1245.0, ft[0:m_r, :],
                            Alu.mult, Alu.add,
                        )
                        # v = S*2^-11 - (0.5 - 2^-12)
                        vt = v_pool.tile([124, N_CHUNK], dt.float32)
                        nc.vector.tensor_scalar(
                            vt[0:m_r, :], st[0:m_r, :], 0.00048828125,
                            0.499755859375, Alu.mult, Alu.subtract,
                        )
                        # r = RNE(v) = res - 5
                        rt = r_pool.tile([124, N_CHUNK], dt.float32)
                        nc.vector.tensor_scalar(
                            rt[0:m_r, :], vt[0:m_r, :], MAGIC, -MAGIC, Alu.add, Alu.add
                        )
                        # clip to [-128, 127] -> int8
                        ot = o_pool.tile([124, N_CHUNK], dt.int8)
                        nc.vector.tensor_scalar(
                            ot[0:m_r, :], rt[0:m_r, :], -128.0, 127.0, Alu.max, Alu.min
                        )
                        nc.sync.dma_start(
                            y_d[f, r0 : r0 + m_r, c * N_CHUNK : (c + 1) * N_CHUNK],
                            ot[0:m_r, :],
                        )

    nc.compile()
    return nc


def _make_in_maps(x: np.ndarray, w: np.ndarray, b: np.ndarray) -> list:
    id4 = (-128.0 * np.eye(124, dtype=np.float32)).astype(np.float16)
    ones2 = np.ones((2, FREE_IN), dtype=np.float32).astype(ml_dtypes.bfloat16)
    in_maps = []
    for core in range(N_CORES):
        lo = core * F_PER_CORE
        hi = lo + F_PER_CORE
        x_shard = np.ascontiguousarray(x[lo:hi]).reshape(F_PER_CORE, H_IN, FREE_IN)
        lhsT = _build_lhsT(w[lo:hi], b[lo:hi])
        in_maps.append({"x": x_shard, "lhsT": lhsT, "id4": id4, "ones2": ones2})
    return in_maps


def kernel(x: np.ndarray, w: np.ndarray, b: np.ndarray) -> np.ndarray:
    """x: int8 [64, 514, 514, 4]; w: int8 [64, 3, 3, 1]; b: int32 [64].

    Returns int8 [64, 512, 512, 4].
    """
    from concourse.bass_utils import run_bass_kernel_spmd

    if "nc" not in _PROGRAM_CACHE:
        _PROGRAM_CACHE["nc"] = _build_program()
    nc = _PROGRAM_CACHE["nc"]

    F = x.shape[0]
    assert F == N_CORES * F_PER_CORE

    in_maps = _make_in_maps(x, w, b)
    res = run_bass_kernel_spmd(nc, in_maps, core_ids=list(range(N_CORES)))

    out = np.empty((F, H_OUT, 512, D), dtype=np.int8)
    for core in range(N_CORES):
        lo = core * F_PER_CORE
        y = res.results[core]["y"]  # [8, 512, 2048] int8
        out[lo : lo + F_PER_CORE] = y.reshape(F_PER_CORE, H_OUT, 512, D)
    return out



# revision 3
# speedup vs baseline: 1.5643x; 1.5643x over previous
"""Trainium2 Bass kernel for nn_Conv2D_80796924772741.

Depthwise (grouped, F=64) 3x3 valid conv over [F, 514, 514, 4] int8 with
per-channel int8 weights + int32 bias, followed by exact fixed-point requant
  res = (acc * 19920 + 2^21) >> 22 ;  out = clip(res - 5, -128, 127) int8
(reduced_mantissa 19920 = 1245 * 16 -> res = (acc*1245 + 2^17) >> 18).

Sharding: F=64 split across 8 NeuronCores (8 channels each), embarrassingly
parallel.

Per-core compute, per (channel, H-window) group ([M<=124 rows, 2048 cols]):
 - PE: conv via Toeplitz-band stationary matmuls over H-windows
   (contraction = input rows; all 3 H-taps in the band diagonals; 3
   matmuls per 512-col chunk for the 3 W-taps, W-shift = +4n free-dim
   offset). Bias b rides two all-ones rhs partitions. PSUM = acc+b exact.
 - ACT1: hi16 = int16(ps * 2^-7 - 0.498046875)   [RNE+sat conversion
   == floor((acc+b)/128); no ties on the 1/128 grid]
 - DVE:  hif  = fp16(hi16)                        [exact, |hi| <= 1168]
 - PE:   ps  += (-128*I) @ hif  -> lo = (acc+b) mod 128 in PSUM
 - ACT2: q16  = int16(ps * 9.7265625 - 0.498046875)  [= floor(lo*1245/128)]
 - DVE:  S32  = hi16 * 1245 + q16   (scalar_tensor_tensor; exact < 2^21)
 - DVE:  out  = int8(S32 * 2^-11 - 4.999755859375)
   [RNE -> floor(S/2^11 + 1/2) - 5 = res - 5; int8 SATURATION == clip]
All intermediates exact in fp32; conversion semantics (RNE+saturate on both
ACT and DVE, fp32-internal ALU) verified on hardware. Bit-exact vs the
int64 reference.
"""

import numpy as np
import ml_dtypes

F_PER_CORE = 8
H_IN = 514
W_IN = 514
D = 4
H_OUT = 512
WD_OUT = 2048  # 512 * 4
FREE_IN = W_IN * D  # 2056
N_CHUNK = 512
N_CORES = 8

# H windows: output rows per window (partition-limited: K = M + 4 <= 128)
WINDOWS = [(0, 124), (124, 124), (248, 124), (372, 124), (496, 16)]


def _build_lhsT(w_core: np.ndarray, b_core: np.ndarray) -> np.ndarray:
    """[128, 8*3*124] bf16 stationary: per (channel, w-tap) a Toeplitz band.

    Layout column block (f*3 + n)*124 : +124  holds T_n for channel f.
    T_n[2 + i + m, i] = w[f, m, n]  (rows 2.. are conv data partitions)
    T_0[0, i] = 8*floor(b/8) ; T_0[1, i] = b mod 8  (bias rows, multiplied
    by all-ones rhs partitions 0/1; both parts bf16-exact).
    """
    out = np.zeros((128, F_PER_CORE * 3 * 124), dtype=np.float32)
    for f in range(F_PER_CORE):
        b_f = int(b_core[f])
        bh = b_f >> 3  # floor division
        bl = b_f - 8 * bh
        for n in range(3):
            base = (f * 3 + n) * 124
            if n == 0:
                out[0, base : base + 124] = float(8 * bh)
                out[1, base : base + 124] = float(bl)
            for m in range(3):
                wv = float(int(w_core[f, m, n, 0]))
                idx = np.arange(124)
                out[2 + idx + m, base + idx] = wv
    return out.astype(ml_dtypes.bfloat16)


_PROGRAM_CACHE = {}


def _build_program():
    import concourse.bass as bass
    import concourse.tile as tile
    from concourse import bacc, mybir

    nc = bacc.Bacc(
        "TRN2", target_bir_lowering=False, debug=False, num_devices=N_CORES
    )
    dt = mybir.dt
    Alu = mybir.AluOpType
    Act = mybir.ActivationFunctionType

    x_d = nc.dram_tensor(
        "x", [F_PER_CORE, H_IN, FREE_IN], dt.int8, kind="ExternalInput"
    ).ap()
    lhsT_d = nc.dram_tensor(
        "lhsT", [128, F_PER_CORE * 3 * 124], dt.bfloat16, kind="ExternalInput"
    ).ap()
    id_d = nc.dram_tensor("id4", [124, 124], dt.float16, kind="ExternalInput").ap()
    ones_d = nc.dram_tensor("ones2", [2, FREE_IN], dt.bfloat16, kind="ExternalInput").ap()
    y_d = nc.dram_tensor(
        "y", [F_PER_CORE, H_OUT, WD_OUT], dt.int8, kind="ExternalOutput"
    ).ap()

    groups = [(f, r0, m_r) for f in range(F_PER_CORE) for (r0, m_r) in WINDOWS]

    with tile.TileContext(nc) as tc:
        with (
            tc.tile_pool(name="const", bufs=1) as const_pool,
            tc.tile_pool(name="xin", bufs=3) as x_pool,
            tc.tile_pool(name="psum", bufs=2, space="PSUM") as psum_pool,
            tc.tile_pool(name="hi16", bufs=2) as hi_pool,
            tc.tile_pool(name="hif", bufs=2) as hif_pool,
            tc.tile_pool(name="q16", bufs=2) as q_pool,
            tc.tile_pool(name="s32", bufs=2) as s_pool,
            tc.tile_pool(name="out8", bufs=3) as o_pool,
        ):
            lhsT_t = const_pool.tile([128, F_PER_CORE * 3 * 124], dt.bfloat16)
            nc.sync.dma_start(lhsT_t[:], lhsT_d[:])
            id_t = const_pool.tile([124, 124], dt.float16)
            nc.sync.dma_start(id_t[:], id_d[:])

            def phase2(f, r0, m_r, ps, hi16, hif):
                # fixup: ps += -128 * hi  -> lo in PSUM
                for c in range(4):
                    nc.tensor.matmul(
                        ps[0:m_r, c * N_CHUNK : (c + 1) * N_CHUNK],
                        id_t[0:m_r, 0:m_r],
                        hif[0:m_r, c * N_CHUNK : (c + 1) * N_CHUNK],
                        start=False,
                        stop=True,
                        skip_group_check=True,
                    )
                # q16 = floor(lo * 1245/128)
                q16 = q_pool.tile([124, WD_OUT], dt.int16)
                nc.scalar.activation(
                    q16[0:m_r, :], ps[0:m_r, :], Act.Copy,
                    bias=-0.498046875, scale=9.7265625,
                )
                # S = hi*1245 + q  (exact, < 2^21)
                s32 = s_pool.tile([124, WD_OUT], dt.int32)
                nc.vector.scalar_tensor_tensor(
                    s32[0:m_r, :], hi16[0:m_r, :], 1245.0, q16[0:m_r, :],
                    Alu.mult, Alu.add,
                )
                # out = sat8(RNE(S*2^-11 - 4.999755859375)) = clip(res-5)
                o8 = o_pool.tile([124, WD_OUT], dt.int8)
                nc.vector.tensor_scalar(
                    o8[0:m_r, :], s32[0:m_r, :], 0.00048828125,
                    -4.999755859375, Alu.mult, Alu.add,
                )
                nc.sync.dma_start(y_d[f, r0 : r0 + m_r, :], o8[0:m_r, :])

            prev = None
            for (f, r0, m_r) in groups:
                k_r = m_r + 4  # 2 ones rows + m_r + 2 data rows
                xt = x_pool.tile([128, FREE_IN], dt.bfloat16)
                # ones rows (bias partitions)
                nc.sync.dma_start(xt[0:2, :], ones_d[:])
                # data rows with int8 -> bf16 cast (SWDGE)
                nc.gpsimd.dma_start(
                    xt[2 : 2 + m_r + 2, :], x_d[f, r0 : r0 + m_r + 2, :]
                )
                ps = psum_pool.tile([124, WD_OUT], dt.float32)
                for n in range(3):
                    base = (f * 3 + n) * 124
                    for c in range(4):
                        nc.tensor.matmul(
                            ps[0:m_r, c * N_CHUNK : (c + 1) * N_CHUNK],
                            lhsT_t[0:k_r, base : base + m_r],
                            xt[0:k_r, c * N_CHUNK + 4 * n : c * N_CHUNK + 4 * n + N_CHUNK],
                            start=(n == 0),
                            stop=False,
                            skip_group_check=True,
                        )
                if prev is not None:
                    phase2(*prev)
                # hi16 = floor((acc+b)/128)  via RNE(x - 0.498046875)
                hi16 = hi_pool.tile([124, WD_OUT], dt.int16)
                nc.scalar.activation(
                    hi16[0:m_r, :], ps[0:m_r, :], Act.Copy,
                    bias=-0.498046875, scale=0.0078125,
                )
                hif = hif_pool.tile([124, WD_OUT], dt.float16)
                nc.vector.tensor_scalar(
                    hif[0:m_r, :], hi16[0:m_r, :], 0, None, Alu.add
                )
                prev = (f, r0, m_r, ps, hi16, hif)
            phase2(*prev)

    nc.compile()
    return nc


def _make_in_maps(x: np.ndarray, w: np.ndarray, b: np.ndarray) -> list:
    id4 = (-128.0 * np.eye(124, dtype=np.float32)).astype(np.float16)
    ones2 = np.ones((2, FREE_IN), dtype=np.float32).astype(ml_dtypes.bfloat16)
    in_maps = []
    for core in range(N_CORES):
        lo = core * F_PER_CORE
        hi = lo + F_PER_CORE
        x_shard = np.ascontiguousarray(x[lo:hi]).reshape(F_PER_CORE, H_IN, FREE_IN)
        lhsT = _build_lhsT(w[lo:hi], b[lo:hi])
        in_maps.append({"x": x_shard, "lhsT": lhsT, "id4": id4, "ones2": ones2})
    return in_maps


def kernel(x: np.ndarray, w: np.ndarray, b: np.ndarray) -> np.ndarray:
    """x: int8 [64, 514, 514, 4]; w: int8 [64, 3, 3, 1]; b: int32 [64].

    Returns int8 [64, 512, 512, 4].
    """
    from concourse.bass_utils import run_bass_kernel_spmd

    if "nc" not in _PROGRAM_CACHE:
        _PROGRAM_CACHE["nc"] = _build_program()
    nc = _PROGRAM_CACHE["nc"]

    F = x.shape[0]
    assert F == N_CORES * F_PER_CORE

    in_maps = _make_in_maps(x, w, b)
    res = run_bass_kernel_spmd(nc, in_maps, core_ids=list(range(N_CORES)))

    out = np.empty((F, H_OUT, 512, D), dtype=np.int8)
    for core in range(N_CORES):
        lo = core * F_PER_CORE
        y = res.results[core]["y"]  # [8, 512, 2048] int8
        out[lo : lo + F_PER_CORE] = y.reshape(F_PER_CORE, H_OUT, 512, D)
    return out
